# revision 8
# baseline (speedup 1.0000x reference)
"""AdaptiveFocalLoss on 8 TRN2 NeuronCores (Bass/Tile).

Data-parallel over batch N (8 images -> 8 cores). Host-side prep is
layout + indexing only: position-major fp8(e3m4) logits (channel
innermost), a gather of the target-class logit (fp16, with the
Schraudolph log bias pre-folded: xt' = logits[target] + 10.3574873),
and the per-class alpha table (global bincount) broadcast to
alpha_pos = alpha[target] (fp16).  The exp/log-sum/focal math all
stays on device.

Per-core device computation (positions P = 262144 = 128 x 2048, C = 16):
  layout: x [128, 2048*16] fp8e3, partition p holds positions
          p*2048..p*2048+2047, channel innermost.
  ex   = exp(x)                     (ACT, fp16 out; the only ACT work)
  D    = sum_c ex                   (DVE pairwise tree over the
                                     innermost 16; packed views -> 2x)
  nlp  = ln(D) - xt                 one STT: bitcast(D) * ln2/1024 - xt'
                                    (Schraudolph log, mean-centered via
                                     the bias folded into xt')
  p    = exp(-nlp)                  Schraudolph: uint16 code =
                                    nlp*(-1477.32)+15301.09, bitcast fp16
  u=1-p; v=u*u; w=v*nlp
  loss_partial = sum(w * alpha_pos) (STT with accum_out)
The first epilogue block runs on the otherwise-idle Pool engine; the
rest on DVE.  No tensor-engine work, no PSUM, no collectives: per-core
partial sums are combined on host, loss = total / (numel + eps).
"""

import sys

sys.path.insert(0, "/opt/trn_rl_repo")

import numpy as np
import ml_dtypes

import bass_rust as _bass_rust
import concourse.bass as bass
import concourse.bacc as bacc
import concourse.tile as tile
from concourse import mybir
from concourse.bass_utils import run_bass_kernel_spmd
from concourse.hw_specs import get_activation_tables


class _Bacc(bacc.Bacc):
    def insert_act_table_loads(self):
        # Only Exp is used; keep it resolvable only via the combined
        # natural_log_exp set so a single ACT_TABLE_LOAD serves the whole
        # kernel (set ids must stay aligned with act_info.json, so filter
        # set contents instead of reordering).
        has_activation = any(
            isinstance(i, mybir.InstActivation)
            for b in self.main_func.blocks
            for i in b.instructions
        )
        if not has_activation:
            return
        AFT = mybir.ActivationFunctionType
        tables = []
        for name, fns in get_activation_tables(self.m.arch).items():
            if name != "natural_log_exp_and_others":
                fns = fns - {AFT.Exp, AFT.Ln}
            tables.append((name, fns))
        _bass_rust.insert_act_table_loads(self, tables)


# ---- problem constants (hardcoded; kernel.py must be self-contained) ----
N, C, H, W = 8, 16, 512, 512
POS = H * W              # positions per core = 262144
PPART = POS // 128       # positions per partition = 2048

GAMMA = 2.0
SMOOTH = 1e-8
ALPHA_SMOOTH = 0.1

FP32 = mybir.dt.float32
FP16 = mybir.dt.float16
FP8 = mybir.dt.float8e3          # e3m4 <-> ml_dtypes.float8_e3m4
U16 = mybir.dt.uint16
I16 = mybir.dt.int16
AX = mybir.AxisListType
OP = mybir.AluOpType
AF = mybir.ActivationFunctionType

# Schraudolph fp16 exp: code = round(nlp * SCH_MUL + SCH_ADD), bitcast.
SCH_MUL = -1024.0 / float(np.log(2.0))       # -1477.3197
SCH_ADD = 15301.087                          # (15 - 0.0575322)*1024
# Schraudolph fp16 log: ln(D) = bits(D) * LOG_SCALE - 10.3574873, the
# constant is folded into xt on the host.
LOG_SCALE = float(np.log(2.0)) / 1024.0      # 6.769015e-4

# exp/DMA groups (positions-per-partition): small first so the exp
# stream starts early, small last to shorten the drain tail.
GROUPS = [128, 256, 512, 512, 384, 128, 128]
assert sum(GROUPS) == PPART
# channel-sum trees merge adjacent groups (indices into GROUPS).
TREE_UNITS = [(0, 1), (2,), (3,), (4,), (5, 6)]
# epilogue blocks (positions).  Pool can't run these (its ISA lacks
# the tensor-scalar-ptr opcodes) so they all stay on DVE.
EPI = [1024, 512, 256, 128, 128]
assert sum(EPI) == PPART
POOL_EPI = set()


def build_nc(compile_graph=True):
    nc = _Bacc("TRN2", target_bir_lowering=False, debug=False,
               num_devices=8)

    x_ext = nc.declare_dram_parameter("x", [128, PPART * C], FP8,
                                      isOutput=False)
    xt_ext = nc.declare_dram_parameter("xt", [128, PPART], FP16,
                                       isOutput=False)
    al_ext = nc.declare_dram_parameter("al", [128, PPART], FP16,
                                       isOutput=False)
    out_ext = nc.declare_dram_parameter("out", [128, len(EPI)], FP32,
                                        isOutput=True)

    with tile.TileContext(nc) as tc:
        with (
            tc.tile_pool(name="singles", bufs=1) as singles,
            tc.tile_pool(name="tree", bufs=2) as tree,
            tc.tile_pool(name="blk", bufs=2) as blk,
        ):
            warm_in = singles.tile([128, 1], FP16)
            warm_out = singles.tile([128, 1], FP16)
            xbuf = singles.tile([128, PPART * C], FP8)
            exbuf = singles.tile([128, PPART * C], FP16)
            dbuf = singles.tile([128, PPART], FP16)
            nlp = singles.tile([128, PPART], FP16)
            xt = singles.tile([128, PPART], FP16)
            al = singles.tile([128, PPART], FP16)
            loss_col = singles.tile([128, len(EPI)], FP32)

            # pre-warm: materialize the activation bias const + table
            # load before the bulk DMAs swamp the queues.
            nc.vector.memset(warm_in, 0.0)
            nc.scalar.activation(out=warm_out, in_=warm_in, func=AF.Exp)

            gstarts = np.cumsum([0] + GROUPS)
            for g, gp in enumerate(GROUPS):
                dma_eng = nc.sync if g % 2 == 0 else nc.gpsimd
                c0 = int(gstarts[g]) * C
                dma_eng.dma_start(out=xbuf[:, c0:c0 + gp * C],
                                  in_=x_ext[:, c0:c0 + gp * C])
                if g == 2:
                    nc.gpsimd.dma_start(out=xt, in_=xt_ext[:, :])
                if g == 3:
                    nc.gpsimd.dma_start(out=al, in_=al_ext[:, :])

            def emit_exp(g):
                c0 = int(gstarts[g]) * C
                cw = GROUPS[g] * C
                nc.scalar.activation(out=exbuf[:, c0:c0 + cw],
                                     in_=xbuf[:, c0:c0 + cw], func=AF.Exp)

            def emit_tree(unit):
                p0 = int(gstarts[unit[0]])
                gp = int(sum(GROUPS[g] for g in unit))
                ex3 = exbuf[:, p0 * C:(p0 + gp) * C].rearrange(
                    "p (f c) -> p f c", c=C)
                l1 = tree.tile([128, gp, 8], FP16, tag="l1")
                nc.vector.tensor_add(l1, ex3[:, :, 0:8], ex3[:, :, 8:16])
                l2 = tree.tile([128, gp, 4], FP16, tag="l2")
                nc.vector.tensor_add(l2, l1[:, :, 0:4], l1[:, :, 4:8])
                l3 = tree.tile([128, gp, 2], FP16, tag="l3")
                nc.vector.tensor_add(l3, l2[:, :, 0:2], l2[:, :, 2:4])
                nc.vector.tensor_add(dbuf[:, p0:p0 + gp],
                                     l3[:, :, 0:1].squeeze(),
                                     l3[:, :, 1:2].squeeze())

            estarts = np.cumsum([0] + EPI)

            def emit_epi(b):
                eng = nc.gpsimd if b in POOL_EPI else nc.vector
                cols = slice(int(estarts[b]), int(estarts[b + 1]))
                bp = EPI[b]
                eng.scalar_tensor_tensor(
                    out=nlp[:, cols], in0=dbuf[:, cols].bitcast(I16),
                    scalar=LOG_SCALE, in1=xt[:, cols],
                    op0=OP.mult, op1=OP.subtract)
                pc = blk.tile([128, bp], U16, tag=f"pc{eng is nc.gpsimd}")
                eng.tensor_scalar(out=pc, in0=nlp[:, cols],
                                  scalar1=SCH_MUL, scalar2=SCH_ADD,
                                  op0=OP.mult, op1=OP.add)
                u_t = blk.tile([128, bp], FP16, tag=f"u{eng is nc.gpsimd}")
                eng.tensor_scalar(out=u_t, in0=pc.bitcast(FP16),
                                  scalar1=-1.0, scalar2=1.0,
                                  op0=OP.mult, op1=OP.add)
                v_t = blk.tile([128, bp], FP16, tag=f"v{eng is nc.gpsimd}")
                eng.tensor_mul(v_t, u_t, u_t)
                w_t = blk.tile([128, bp], FP16, tag=f"w{eng is nc.gpsimd}")
                eng.tensor_mul(w_t, v_t, nlp[:, cols])
                f_t = blk.tile([128, bp], FP16, tag=f"f{eng is nc.gpsimd}")
                eng.scalar_tensor_tensor(
                    out=f_t, in0=w_t, scalar=1.0, in1=al[:, cols],
                    op0=OP.mult, op1=OP.mult,
                    accum_out=loss_col[:, b:b + 1])

            # tree units complete at these exp-group indices
            unit_ready = {u: u[-1] for u in TREE_UNITS}
            # epilogue block b is covered once dbuf[0:estarts[b+1]] exists;
            # emit it right after the tree unit that completes it.
            epi_after = {}
            covered = 0
            for ui, u in enumerate(TREE_UNITS):
                covered += sum(GROUPS[g] for g in u)
                for b in range(len(EPI)):
                    if b not in [x for v in epi_after.values() for x in v]:
                        if int(estarts[b + 1]) <= covered:
                            epi_after.setdefault(ui, []).append(b)

            with nc.allow_low_precision("fp16 tree sums, rel err ~1e-3"):
                for g in range(len(GROUPS)):
                    emit_exp(g)
                    for ui, u in enumerate(TREE_UNITS):
                        if unit_ready[u] == g:
                            emit_tree(u)
                            for b in epi_after.get(ui, []):
                                emit_epi(b)

            nc.sync.dma_start(out=out_ext[:, :], in_=loss_col)

    if compile_graph:
        nc.compile()
    return nc


_CACHED = {}


def _get_nc():
    if "nc" not in _CACHED:
        _CACHED["nc"] = build_nc()
    return _CACHED["nc"]


def make_in_maps(logits, target):
    logits = np.asarray(logits, dtype=np.float32)
    target = np.asarray(target).astype(np.int64)

    # adaptive alpha from the global class histogram
    counts = np.bincount(target.reshape(-1), minlength=C).astype(np.float64)
    total = float(target.size)
    freq = counts / total
    w = 1.0 / (freq + ALPHA_SMOOTH)
    present = counts > 0
    wsum = np.sum(np.where(present, w, 0.0))
    alpha = np.where(present, w / wsum, 1.0)

    # position-major, channel-innermost fp8 layout
    x8 = logits.astype(ml_dtypes.float8_e3m4)          # (N, C, H, W)
    xpos = np.ascontiguousarray(x8.transpose(0, 2, 3, 1))   # (N, H, W, C)
    xpos = xpos.reshape(N, 128, PPART * C)

    tflat = target.reshape(N, POS)
    xt = np.take_along_axis(logits.reshape(N, C, POS), tflat[:, None],
                            axis=1)[:, 0]              # (N, POS) fp32
    # fold the Schraudolph-log bias into xt: nlp = bits*scale - xt'
    xt = (xt + 10.3574873).astype(np.float16).reshape(N, 128, PPART)
    al = alpha[tflat].astype(np.float16).reshape(N, 128, PPART)

    in_maps = []
    for n in range(N):
        in_maps.append({
            "x": xpos[n],
            "xt": xt[n],
            "al": al[n],
        })
    return in_maps


def combine(results):
    total = 0.0
    for r in results:
        total += np.asarray(r["out"], dtype=np.float64).sum()
    loss = total / (float(N * POS) + SMOOTH)
    return np.float32(loss)


def kernel(logits, target, trace=False, **run_kwargs):
    nc = _get_nc()
    in_maps = make_in_maps(logits, target)
    res = run_bass_kernel_spmd(nc, in_maps, core_ids=list(range(8)),
                               trace=trace, **run_kwargs)
    out = combine(res.results)
    if trace:
        kernel.last_result = res
    return out
